# revision 9
# baseline (speedup 1.0000x reference)
"""Betti-matching-loss preprocessing kernel for 8 TRN2 NeuronCores.

Reference computation (per full input of shape (B=4, C=1, D=128, H=256, W=256)):
    pred_super   = 1 - maxpool3d_2x(sigmoid(input))   -> sigmoid is monotone, so
                 = sigmoid(-maxpool3d_2x(input))
    target_super = 1 - (maxpool3d_2x(target) > 0.5)   = (maxpool3d_2x(target) <= 0.5)
    out = stack([pred_super, target_super])           # (2, B, C, 64, 128, 128)

Sharding: pure data parallel. 8 shards = 4 batch samples x 2 D-halves of 64
planes each (the D split at an even index never crosses a pool window).

Per-core kernel: the run is SDMA-engine-busy bound (16 engines ~92% busy;
~7 cycle overhead per <=4KB packet), so the layout maximizes contiguity:
partition (a, jh) of a load tile receives rows 8*jh..8*jh+7 of planes
2a/2a+1 -- 8 KB contiguous per descriptor (4 KB packets) vs 2 KB in the
row-pair layout.  The pool tree stays 3 DVE tensor_max ops on all 128
partitions: D (plane pair, free dim), H (row pairs), W (column pairs),
leaving 4 consecutive output rows per partition -- 2 KB contiguous stores
vs 512 B.  Stores issue on the ACT HWDGE ring as soon as each chunk's
result is ready; the last two (half-size) chunks' stores go to the Sync
ring, which is idle once the final load has been triggered, shortening
the drain-down chain.
"""

import numpy as np

import bass_rust
import concourse.bass as bass
import concourse.mybir as mybir
import concourse.tile as tile
from concourse.bass_utils import run_bass_kernel_spmd
from concourse.vector_clock import ScopedClock

f32 = mybir.dt.float32


def _patched_drain_and_barrier(self, tick_clock, wait_clock):
    """Replacement for TileContext._drain_and_barrier.

    The stock version hangs every outstanding semaphore wait on one Drain
    instruction; the walrus in this environment rejects >1 sync-wait per
    non-EventSemaphore instruction ("Too many sync wait commands").  Emit
    one sequencer NOP per semaphore wait instead, then drain + barrier.
    """
    ((_, vclock),) = ScopedClock({None: tick_clock.global_clock}).items()
    ticks = list(vclock)
    for proc_idx, sem in self.sems.allocated().items():
        t = ticks[proc_idx]
        if t > 0:
            self.nc.sync.nop()._wait_ge(sem, bass_rust.tick_to_sem(t, proc_idx))
    self.nc.sync.drain()
    self.nc.all_engine_barrier(sem_only=True)
    popped = self.nc._tile_sem_poison_stack.pop()
    assert popped is self._sem_poison
    self.nc.clear_and_free_semaphores(list(self.sems.allocated().values()))


tile.TileContext._drain_and_barrier = _patched_drain_and_barrier


def _split_excess_waits(nc: bass.Bass) -> None:
    """Walrus in this env caps sync-waits at 1 per instruction (2 for
    EventSemaphore).  Move excess waits onto same-engine NoOps inserted
    immediately before the offending instruction."""
    for f in nc.m.functions:
        for bb in f.blocks:
            insts = bb.instructions
            out = []
            changed = False
            for inst in insts:
                si = inst.sync_info
                cap = 2 if type(inst).__name__ == "InstEventSemaphore" else 1
                if si is not None and len(si.on_wait) > cap:
                    w = list(si.on_wait)
                    for k, extra in enumerate(w[cap:]):
                        nop = mybir.InstNoOp(
                            name=f"{inst.name}-xw{k}",
                            engine=inst.engine,
                            sync_info=mybir.SyncInfo(
                                on_wait=[extra], on_update=[]
                            ),
                            bass_nofuse=True,
                        )
                        nc.register_instruction(nop, overwrite=True)
                        out.append(nop)
                    inst.sync_info = mybir.SyncInfo(
                        on_wait=w[:cap], on_update=si.on_update
                    )
                    changed = True
                out.append(inst)
            if changed:
                bb.instructions = out

B, C, D, H, W = 4, 1, 128, 256, 256
NCORES = 8
D_SH = D // 2      # 64 input planes per core
DZ = D_SH // 2     # 32 output planes per core
HO, WO = H // 2, W // 2
PPT = 8            # input planes per full load tile (2 MB DMAs)


def build_nc(d_sh: int = D_SH, ppt: int = PPT) -> bass.Bass:
    nt = d_sh // ppt       # full-size load tiles per tensor
    dz = d_sh // 2
    nc = bass.Bass()
    inp = nc.declare_dram_parameter("input", [d_sh, H, W], f32, isOutput=False)
    tgt = nc.declare_dram_parameter("target", [d_sh, H, W], f32, isOutput=False)
    out = nc.declare_dram_parameter("out", [2, dz, HO, WO], f32, isOutput=True)

    # chunk schedule: full tiles, last full tile split in half to shorten
    # the final compute drain-down
    chunks = [(q * ppt, ppt) for q in range(nt - 1)]
    last = (nt - 1) * ppt
    if ppt >= 8:
        chunks += [(last, ppt // 2), (last + ppt // 2, ppt // 2)]
    else:
        chunks += [(last, ppt)]

    n_g = 2 * len(chunks)  # one g tile per (chunk, tensor), all kept live
    with tile.TileContext(nc) as tc:
        with (
            tc.tile_pool(name="load", bufs=7) as load_pool,
            tc.tile_pool(name="lvl1", bufs=3) as pool1,
            tc.tile_pool(name="lvl2", bufs=3) as pool2,
            tc.tile_pool(name="lvl3", bufs=3) as pool3,
            tc.tile_pool(name="post", bufs=n_g) as pool4,
        ):
            for ci, (d0, cs) in enumerate(chunks):
                # partition (a, jh): a = plane pair, jh = row octet; RR input
                # rows of 8KB (full) / 4KB (half) contiguity per descriptor
                A = cs // 2            # plane pairs = output planes
                JH = 128 // A          # row groups per plane
                RR = H // JH           # input rows per group (8 or 4)
                M = RR // 2            # output rows per partition per plane
                for which, src in ((0, inp), (1, tgt)):
                    # ---- load: partition (a,jh) <- rows RR*jh..+RR-1 of
                    # planes 2a and 2a+1 (free dim: pl, then RR*W contig) ----
                    # The host hands each DRAM tensor plane-permuted to
                    # (pl, pair): planes [0,2,..,62, 1,3,..,63].  Partition
                    # (a, jh) then has uniform DRAM stride (pair step ==
                    # JH row-group steps), so one 3-dim AP covers the load.
                    t = load_pool.tile([128, ppt * 512], f32, tag="load")
                    sv = src.rearrange(
                        "(pl m) (jh rr) w -> (m jh) pl (rr w)", pl=2, rr=RR
                    )[(d0 // 2) * JH:(d0 // 2) * JH + 128]
                    dv = t[:, :2 * RR * W].rearrange(
                        "p (pl rw) -> p pl rw", pl=2
                    )
                    nc.sync.dma_start(dv, sv)

                    # ---- level 1: pool D (plane 2a vs 2a+1, free halves) ----
                    # (this walrus only codegens TensorTensor on DVE)
                    u = pool1.tile([128, (ppt // 2) * 512], f32, tag="u")
                    tv = t[:, :2 * RR * W].rearrange(
                        "p (pl rw) -> p pl rw", pl=2
                    )
                    nc.vector.tensor_max(u[:, :RR * W], tv[:, 0], tv[:, 1])

                    # ---- level 2: pool H (row 2m vs 2m+1 within octet) ----
                    v = pool2.tile([128, (ppt // 2) * 256], f32, tag="v")
                    uv = u[:, :RR * W].rearrange(
                        "p (m hh w) -> p m hh w", hh=2, w=W
                    )
                    nc.vector.tensor_max(
                        v[:, :M * W].rearrange("p (m w) -> p m w", w=W),
                        uv[:, :, 0],
                        uv[:, :, 1],
                    )

                    # ---- level 3: pool W (even/odd columns) ----
                    o = pool3.tile([128, (ppt // 2) * 128], f32, tag="o")
                    vv = v[:, :M * W].rearrange(
                        "p (m wo two) -> p m wo two", wo=WO, two=2
                    )
                    nc.vector.tensor_max(
                        o[:, :M * WO].rearrange("p (m wo) -> p m wo", wo=WO),
                        vv[:, :, :, 0],
                        vv[:, :, :, 1],
                    )

                    # ---- pointwise ----
                    g = pool4.tile([128, (ppt // 2) * 128], f32, tag="g")
                    if which == 0:
                        nc.scalar.activation(
                            g[:, :M * WO], o[:, :M * WO],
                            mybir.ActivationFunctionType.Sigmoid,
                            bias=0.0, scale=-1.0,
                        )
                    else:
                        nc.vector.tensor_scalar(
                            g[:, :M * WO], o[:, :M * WO],
                            0.5, None, mybir.AluOpType.is_le,
                        )

                    # ---- store: partition (a,jh) -> rows M*jh..+M-1 of
                    # output plane z0+a (2KB/1KB contiguous) ----
                    z0 = d0 // 2
                    dst = out[which, z0:z0 + A].rearrange(
                        "z (jh rr) w -> (z jh) (rr w)", rr=M
                    )
                    # tail chunks store on the Sync ring (idle after the
                    # last load trigger); the rest on the ACT ring
                    eng = nc.sync if ci >= len(chunks) - 2 else nc.scalar
                    eng.dma_start(dst, g[:, :M * WO])
    _split_excess_waits(nc)
    return nc


_NC_CACHE: dict = {}


def perm_planes(x: np.ndarray) -> np.ndarray:
    """Even planes then odd planes -- matches the kernel's load AP."""
    return np.ascontiguousarray(np.concatenate([x[0::2], x[1::2]], axis=0))


def kernel(input: np.ndarray, target: np.ndarray) -> np.ndarray:
    input = np.asarray(input, dtype=np.float32)
    target = np.asarray(target, dtype=np.float32)
    assert input.shape == (B, C, D, H, W), input.shape

    if "nc" not in _NC_CACHE:
        _NC_CACHE["nc"] = build_nc()
    nc = _NC_CACHE["nc"]

    in_maps = []
    for i in range(NCORES):
        b, half = divmod(i, 2)
        sl = slice(half * D_SH, (half + 1) * D_SH)
        in_maps.append({
            "input": perm_planes(input[b, 0, sl]),
            "target": perm_planes(target[b, 0, sl]),
        })

    res = run_bass_kernel_spmd(nc, in_maps, core_ids=list(range(NCORES))).results

    full = np.empty((2, B, C, D // 2, HO, WO), dtype=np.float32)
    for i in range(NCORES):
        b, half = divmod(i, 2)
        full[:, b, 0, half * DZ:(half + 1) * DZ] = res[i]["out"]
    return full
